# revision 58
# baseline (speedup 1.0000x reference)
"""Trainium2 Bass kernel for nn_CustomMHA (sparse head-gathered MHA).

Math (reference):
  q/k/v projections of x (all heads) and x1/x2/x3 (one head, head_idx),
  heads gathered: out head i = special (x1/x2/x3 @ W[head_idx]) when
  permutation[i]==0 else x @ W[head_idxs[permutation[i]-1]]; full T x T
  softmax attention per head; concat heads; out = y @ W_proj.T + b_proj.

Strategy (8 NeuronCores, SPMD, no collectives):
  - Head-parallel: 16 "slots" (8 cores x 2). Normal (x-sourced) output head
    positions are resolved on the host into per-slot weight slices
    (gather/permutation is free: it is just weight-row selection). Unused
    slots get zero weights (they compute exp(0) softmax of zeros -> y=0 and
    contribute nothing).
  - Launch A (per core): projections for its 2 slots packed M=128,
    attention with transposed scores (kt on partitions -> softmax sum via
    an extra ones-column on v), partial c_proj into out.T, plus this
    core's 512-token row-chunk of the special head's q/k/v.
  - Host: gathers special q/k/v chunks, then Launch B computes the special
    head's attention qt-sharded across the 8 cores + its c_proj partial.
  - Host sums the partial out.T contributions (c_proj is linear in heads).

All matmuls run as float32r (full-rate fp32 PE mode, free dim 512).
"""

import numpy as np
from contextlib import ExitStack

import concourse.bass as bass
import concourse.tile as tile
from concourse import bacc, mybir
from concourse.bass_utils import run_bass_kernel_spmd

F32 = mybir.dt.float32
F32R = mybir.dt.float32r
AF = mybir.ActivationFunctionType

B, T, C, H, D = 2, 2048, 1024, 16, 64
NC = 8
NT = B * T            # 4096 tokens total, col index = b*T + t
P = 128
NCT = C // P          # 8 contraction tiles
TCH = 512             # projection token chunk
NTCH = NT // TCH      # 8
QCH = 1024            # attention query chunk
NKT = T // P          # 16 key tiles per batch
SCALE = 1.0 / np.sqrt(D)


def _build_launch_a():
    nc = bacc.Bacc("TRN2", target_bir_lowering=False, debug=False, num_devices=NC)

    def din(name, shape, dt=F32):
        return nc.dram_tensor(name, shape, dt, kind="ExternalInput").ap()

    def dout(name, shape, dt=F32):
        return nc.dram_tensor(name, shape, dt, kind="ExternalOutput").ap()

    XT = din("XT", [C, NT], F32R)         # x.T
    X1C = din("X1C", [C, TCH], F32R)      # per-core column chunk of x1.T
    X2C = din("X2C", [C, TCH], F32R)
    X3C = din("X3C", [C, TCH], F32R)
    WQ = din("WQ", [C, P], F32R)          # [c, slot*64+d] packed q weights (transposed)
    WK = din("WK", [C, P], F32R)
    WV = din("WV", [C, P], F32R)
    WSQ = din("WSQ", [C, D], F32R)        # special head weight slices (transposed)
    WSK = din("WSK", [C, D], F32R)
    WSV = din("WSV", [C, D], F32R)
    WP = din("WP", [P, C], F32R)          # c_proj lhsT: [slot*64+d, C_out]
    EYE2 = din("EYE2", [P, D], F32R)      # [I64; I64] stacked identities
    ONES = din("ONES", [P, D + 1], F32R)
    BIAS = din("BIAS", [P, 3])            # per-slot q/k/v biases (cols 0,1,2)
    SBIAS = din("SBIAS", [D, 3])          # special head q/k/v biases

    OUTT = dout("OUTT", [C, NT])          # partial out.T
    QST = dout("QST", [D, TCH])           # special head q/k/v row-chunks
    KST = dout("KST", [D, TCH])
    VST = dout("VST", [D, TCH])

    with tile.TileContext(nc) as tc, ExitStack() as ctx:
        consts = ctx.enter_context(tc.tile_pool(name="consts", bufs=1))
        big = ctx.enter_context(tc.tile_pool(name="big", bufs=1))
        xcp = ctx.enter_context(tc.tile_pool(name="xcp", bufs=4))
        xp = ctx.enter_context(tc.tile_pool(name="xp", bufs=10))
        expp = ctx.enter_context(tc.tile_pool(name="expp", bufs=7))
        stg = ctx.enter_context(tc.tile_pool(name="stg", bufs=6))
        ysp = ctx.enter_context(tc.tile_pool(name="ysp", bufs=4))
        rcp = ctx.enter_context(tc.tile_pool(name="rcp", bufs=1))
        # PSUM budget: 8 banks = att 2x2 + y 1x2 + shared(proj/tp/cp) 2x1
        pp = ctx.enter_context(tc.tile_pool(name="pp", bufs=2, space="PSUM"))
        pa = ctx.enter_context(tc.tile_pool(name="pa", bufs=2, space="PSUM"))
        py_ = ctx.enter_context(tc.tile_pool(name="py", bufs=1, space="PSUM"))

        # ---- constants into SBUF
        def load_w(ap_dram, m, engine=None):
            # [C, m] dram -> [128, NCT*m] sbuf, block ci = contraction tile
            t = consts.tile([P, NCT * m], F32R, tag=f"w{ap_dram.tensor.name}")
            (engine or nc.sync).dma_start(
                t[:].rearrange("p (a m) -> p a m", a=NCT),
                ap_dram.rearrange("(a p) m -> p a m", p=P),
            )
            return t

        wq = load_w(WQ, P, engine=nc.gpsimd)
        wk = load_w(WK, P, engine=nc.gpsimd)
        wv = load_w(WV, P, engine=nc.gpsimd)
        bias = consts.tile([P, 3], F32, tag="bias")
        nc.sync.dma_start(bias[:], BIAS[:])
        # loaded off the critical path (scalar DGE queue): only needed by
        # phase_v / phase_a / phase_c / phase_s
        eye2 = consts.tile([P, D], F32R, tag="eye2")
        nc.scalar.dma_start(eye2[:], EYE2[:])
        ones = consts.tile([P, D + 1], F32R, tag="ones")
        nc.scalar.dma_start(ones[:], ONES[:])
        wp = consts.tile([P, C], F32R, tag="wp")
        nc.scalar.dma_start(wp[:], WP[:])
        wsq = load_w(WSQ, D, engine=nc.scalar)
        wsk = load_w(WSK, D, engine=nc.scalar)
        wsv = load_w(WSV, D, engine=nc.scalar)
        sbias = consts.tile([D, 3], F32, tag="sbias")
        nc.scalar.dma_start(sbias[:], SBIAS[:])

        qT2 = big.tile([P, NT], F32R, tag="qT2")
        kT2 = big.tile([P, NT], F32R, tag="kT2")
        vT2 = big.tile([P, NT], F32R, tag="vT2")
        vaug = big.tile([P, 65 * B * 2 * NKT], F32R, tag="vaug")
        yT2 = big.tile([P, NT], F32R, tag="yT2")

        # ---- projections (slot-packed) for one 512-token chunk
        def phase_p(tj):
            sl = bass.ts(tj, TCH)
            xts = []
            for ci in range(NCT):
                xt = xp.tile([P, TCH], F32R, tag="xt")
                nc.sync.dma_start(xt[:], XT[ci * P:(ci + 1) * P, sl])
                xts.append(xt)
            for w, dst, bcol in ((wq, qT2, 0), (wk, kT2, 1), (wv, vT2, 2)):
                ps = pp.tile([P, TCH], F32, tag="proj")
                for ci in range(NCT):
                    nc.tensor.matmul(
                        ps[:], w[:, ci * P:(ci + 1) * P], xts[ci][:],
                        start=(ci == 0), stop=(ci == NCT - 1),
                    )
                nc.vector.tensor_scalar_add(
                    dst[:, sl], ps[:], bias[:, bcol:bcol + 1]
                )

        # ---- v_aug weight tiles [v | ones] (M=65) per kt, one (batch, slot)
        def phase_v(b, h):
            for k in range(NKT):
                base = 65 * ((b * 2 + h) * NKT + k)
                tp = pp.tile([P, D], F32R, tag="proj")
                nc.tensor.transpose(
                    tp[:, 0:D],
                    vT2[h * D:(h + 1) * D, b * T + k * P: b * T + (k + 1) * P],
                    eye2[h * D:(h + 1) * D, 0:D],
                )
                nc.vector.tensor_copy(vaug[:, base:base + D], tp[:, 0:D])
                nc.vector.tensor_copy(
                    vaug[:, base + D:base + D + 1], ones[:, 0:1]
                )

        # ---- attention for one (b, slot, qchunk)
        # Both slots accumulate y-psum@0 (y@0..63, sum@64).  The full y_aug
        # block is copied to SBUF right away (frees the single y-psum buf for
        # the next iteration); normalization runs from SBUF against the psum
        # bcast of 1/sum (DVE may read only one psum operand).  slot1's rows
        # are partition-shifted into yT2[64:128] by an SBUF->SBUF DMA.
        def phase_a(b, h, qs0, w=QCH):
            hs = slice(h * D, (h + 1) * D)
            qs = b * T + qs0
            nh = w // 512
            yp = py_.tile([P, QCH], F32, tag="y")
            exs = []
            for k in range(NKT):
                ap_ = pa.tile([P, QCH], F32, tag="att")
                for hf in range(nh):
                    nc.tensor.matmul(
                        ap_[:, hf * 512:(hf + 1) * 512],
                        kT2[hs, b * T + k * P: b * T + (k + 1) * P],
                        qT2[hs, qs + hf * 512: qs + (hf + 1) * 512],
                        start=True, stop=True,
                    )
                ex = expp.tile([P, QCH], F32R, tag="exp")
                nc.scalar.activation(
                    ex[:, 0:w], ap_[:, 0:w], AF.Exp, scale=float(SCALE)
                )
                exs.append(ex)
            for k in range(NKT):
                base = 65 * ((b * 2 + h) * NKT + k)
                for hf in range(nh):
                    nc.tensor.matmul(
                        yp[0:65, hf * 512:(hf + 1) * 512],
                        vaug[:, base:base + 65],
                        exs[k][:, hf * 512:(hf + 1) * 512],
                        start=(k == 0), stop=(k == NKT - 1),
                    )
            ys = ysp.tile([P, QCH], F32R, tag="ystage")
            nc.vector.tensor_copy(ys[0:D + 1, 0:w], yp[0:D + 1, 0:w])
            # partition_broadcast on HW reads the tile's physical partition 0
            # (AP partition offsets are ignored), so DMA-shift the sum row to
            # a partition-0 tile before reciprocal + broadcast.
            srow = rcp.tile([P, QCH], F32R, tag="srow")
            nc.gpsimd.dma_start(srow[0:1, 0:w], ys[D:D + 1, 0:w])
            rc = rcp.tile([P, QCH], F32R, tag="rc")
            with nc.allow_low_precision(reason="f32r is fp32 bits"):
                nc.vector.reciprocal(rc[0:1, 0:w], srow[0:1, 0:w])
            bc = rcp.tile([P, QCH], F32R, tag="bc")
            nc.gpsimd.partition_broadcast(bc[0:D, 0:w], rc[0:1, 0:w])
            if h == 0:
                nc.vector.tensor_mul(
                    yT2[0:D, qs:qs + w], ys[0:D, 0:w], bc[0:D, 0:w]
                )
            else:
                ys2 = ysp.tile([P, QCH], F32R, tag="ystage")
                nc.vector.tensor_mul(ys2[0:D, 0:w], ys[0:D, 0:w], bc[0:D, 0:w])
                nc.gpsimd.dma_start(yT2[D:2 * D, qs:qs + w], ys2[0:D, 0:w])

        # ---- partial c_proj -> out.T for one 512-token chunk
        def phase_c(b, qc, use_act=False):
            qs = b * T + qc * TCH
            for cc in range(NCT):
                cp = pp.tile([P, TCH], F32, tag="proj")
                nc.tensor.matmul(
                    cp[:], wp[:, cc * P:(cc + 1) * P], yT2[:, qs:qs + TCH],
                    start=True, stop=True,
                )
                st = stg.tile([P, TCH], F32, tag="stage")
                if use_act and cc % 2 == 1:
                    nc.scalar.copy(st[:], cp[:])
                else:
                    nc.vector.tensor_copy(st[:], cp[:])
                nc.sync.dma_start(
                    OUTT[cc * P:(cc + 1) * P, qs:qs + TCH], st[:]
                )

        # ---- one special-head projection row chunk (q, k or v); off the
        # critical path - emitted late to fill idle PE/DMA during attention.
        def phase_s(j):
            wsb, xc, outd = [(wsq, X1C, QST), (wsk, X2C, KST), (wsv, X3C, VST)][j]
            ps = pp.tile([P, TCH], F32, tag="proj")
            for ci in range(NCT):
                xt = xcp.tile([P, TCH], F32R, tag="xc")
                nc.sync.dma_start(xt[:], xc[ci * P:(ci + 1) * P, :])
                nc.tensor.matmul(
                    ps[0:D, :], wsb[:, ci * D:(ci + 1) * D], xt[:],
                    start=(ci == 0), stop=(ci == NCT - 1),
                )
            st = stg.tile([P, TCH], F32, tag="stage")
            nc.vector.tensor_scalar_add(
                st[0:D, :], ps[0:D, :], sbias[:, j:j + 1]
            )
            nc.sync.dma_start(outd[:], st[0:D, :])

        # Emission order = scheduler priority.  Attention (the ACT-bound
        # critical path) is emitted eagerly; c_proj / special chunks are
        # emitted after, so the greedy scheduler uses them as gap fillers.
        for tj in range(4):
            phase_p(tj)
        phase_v(0, 0)
        phase_v(0, 1)
        phase_a(0, 0, 0)
        phase_a(0, 0, 1024)
        phase_a(0, 1, 0)
        phase_a(0, 1, 1024)
        for tj in range(4, 8):
            phase_p(tj)
        phase_v(1, 0)
        phase_v(1, 1)
        phase_a(1, 0, 0)
        phase_a(1, 0, 1024)
        for qc in range(4):
            phase_c(0, qc)
        for j in range(3):
            phase_s(j)
        phase_a(1, 1, 0)
        phase_c(1, 0, use_act=True)
        phase_c(1, 1, use_act=True)
        phase_a(1, 1, 1024)
        phase_c(1, 2, use_act=True)
        phase_c(1, 3, use_act=True)

    nc.compile()
    return nc


def _build_launch_b():
    nc = bacc.Bacc("TRN2", target_bir_lowering=False, debug=False, num_devices=NC)

    def din(name, shape, dt=F32):
        return nc.dram_tensor(name, shape, dt, kind="ExternalInput").ap()

    QSC = din("QSC", [D, TCH], F32R)  # this core's query chunk (one batch)
    KS = din("KS", [D, T], F32R)      # full keys (that batch)
    VAUG = din("VAUG", [T, D + 1], F32R)  # natural v with ones column appended
    WPS = din("WPS", [D, C], F32R)    # summed special c_proj lhsT
    ONES = din("ONES", [P, D + 1], F32R)
    OUTTS = nc.dram_tensor("OUTTS", [C, TCH], F32, kind="ExternalOutput").ap()

    with tile.TileContext(nc) as tc, ExitStack() as ctx:
        consts = ctx.enter_context(tc.tile_pool(name="consts", bufs=1))
        expp = ctx.enter_context(tc.tile_pool(name="expp", bufs=7))
        stg = ctx.enter_context(tc.tile_pool(name="stg", bufs=4))
        pa = ctx.enter_context(tc.tile_pool(name="pa", bufs=3, space="PSUM"))
        py_ = ctx.enter_context(tc.tile_pool(name="py", bufs=1, space="PSUM"))
        pc = ctx.enter_context(tc.tile_pool(name="pc", bufs=2, space="PSUM"))

        wps = consts.tile([D, C], F32R, tag="wps")
        nc.scalar.dma_start(wps[:], WPS[:])
        ones = consts.tile([P, D + 1], F32R, tag="ones")
        nc.scalar.dma_start(ones[:], ONES[:])
        qs = consts.tile([D, TCH], F32R, tag="qs")
        nc.sync.dma_start(qs[:], QSC[:])
        ks = consts.tile([D, T], F32R, tag="ks")
        for kk in range(4):
            nc.sync.dma_start(
                ks[:, kk * 512:(kk + 1) * 512], KS[:, kk * 512:(kk + 1) * 512]
            )
        vau = consts.tile([P, 65 * NKT], F32R, tag="vau")
        vv = vau[:].rearrange("p (a m) -> p a m", a=NKT)
        vd = VAUG.rearrange("(a p) m -> p a m", p=P)
        for kk in range(4):
            nc.gpsimd.dma_start(
                vv[:, kk * 4:(kk + 1) * 4, :], vd[:, kk * 4:(kk + 1) * 4, :]
            )

        yp = py_.tile([P, TCH], F32, tag="y")
        for k in range(NKT):
            ap_ = pa.tile([P, TCH], F32, tag="att")
            nc.tensor.matmul(
                ap_[:], ks[:, k * P:(k + 1) * P], qs[:], start=True, stop=True
            )
            ex = expp.tile([P, TCH], F32R, tag="exp")
            nc.scalar.activation(ex[:], ap_[:], AF.Exp, scale=float(SCALE))
            nc.tensor.matmul(
                yp[0:65, :], vau[:, 65 * k:65 * k + 65], ex[:],
                start=(k == 0), stop=(k == NKT - 1),
            )
        yst = consts.tile([P, TCH], F32R, tag="yst")
        nc.vector.tensor_copy(yst[0:D + 1, :], yp[0:D + 1, :])
        rc = consts.tile([P, TCH], F32R, tag="rc")
        with nc.allow_low_precision(reason="f32r is fp32 bits"):
            nc.vector.reciprocal(rc[D:D + 1, :], yst[D:D + 1, :])
        bc = pa.tile([P, TCH], F32, tag="att")
        nc.tensor.matmul(
            bc[0:D, :], ones[D:D + 1, 0:D], rc[D:D + 1, :], start=True, stop=True
        )
        ys = consts.tile([D, TCH], F32R, tag="ys")
        nc.vector.tensor_mul(ys[:], yst[0:D, :], bc[0:D, :])
        for cc in range(NCT):
            cp = pc.tile([P, TCH], F32, tag="cp")
            nc.tensor.matmul(
                cp[:], wps[:, cc * P:(cc + 1) * P], ys[:], start=True, stop=True
            )
            st = stg.tile([P, TCH], F32, tag="stage")
            nc.vector.tensor_copy(st[:], cp[:])
            nc.sync.dma_start(OUTTS[cc * P:(cc + 1) * P, :], st[:])

    nc.compile()
    return nc


_CACHE = {}


def _get_nc(which):
    if which not in _CACHE:
        _CACHE[which] = _build_launch_a() if which == "a" else _build_launch_b()
    return _CACHE[which]


def kernel(x1, x2, x3, x, W_attn, b_attn, W_proj, b_proj, head_idx, head_idxs,
           permutation):
    f32 = np.float32
    x1 = np.asarray(x1, f32).reshape(NT, C)
    x2 = np.asarray(x2, f32).reshape(NT, C)
    x3 = np.asarray(x3, f32).reshape(NT, C)
    x = np.asarray(x, f32).reshape(NT, C)
    W_attn = np.asarray(W_attn, f32)
    b_attn = np.asarray(b_attn, f32)
    W_proj = np.asarray(W_proj, f32)
    b_proj = np.asarray(b_proj, f32)
    hidx = int(head_idx)
    head_idxs = np.asarray(head_idxs).astype(np.int64)
    perm = np.asarray(permutation).astype(np.int64)

    xT = np.ascontiguousarray(x.T)
    x1T = np.ascontiguousarray(x1.T)
    x2T = np.ascontiguousarray(x2.T)
    x3T = np.ascontiguousarray(x3.T)

    Wq, Wk, Wv = W_attn[:C], W_attn[C:2 * C], W_attn[2 * C:]
    bq, bk, bv = b_attn[:C], b_attn[C:2 * C], b_attn[2 * C:]

    # output head position -> source head (None = special/x1x2x3 path)
    special_pos = [i for i in range(H) if perm[i] == 0]
    normal = [(i, int(head_idxs[perm[i] - 1])) for i in range(H) if perm[i] != 0]
    slots = normal + [None] * (H - len(normal))

    eye2 = np.vstack([np.eye(D, dtype=f32)] * 2)
    ones_row = np.ones((P, D + 1), f32)
    hs = slice(hidx * D, (hidx + 1) * D)
    wsqT = np.ascontiguousarray(Wq[hs].T)
    wskT = np.ascontiguousarray(Wk[hs].T)
    wsvT = np.ascontiguousarray(Wv[hs].T)
    sbias = np.stack([bq[hs], bk[hs], bv[hs]], axis=1).astype(f32)

    in_maps = []
    for c in range(NC):
        WQ2 = np.zeros((C, P), f32)
        WK2 = np.zeros((C, P), f32)
        WV2 = np.zeros((C, P), f32)
        WP2 = np.zeros((P, C), f32)
        BIAS = np.zeros((P, 3), f32)
        for s in range(2):
            slot = slots[2 * c + s]
            if slot is None:
                continue
            pos, src = slot
            ss = slice(src * D, (src + 1) * D)
            WQ2[:, s * D:(s + 1) * D] = Wq[ss].T
            WK2[:, s * D:(s + 1) * D] = Wk[ss].T
            WV2[:, s * D:(s + 1) * D] = Wv[ss].T
            WP2[s * D:(s + 1) * D, :] = W_proj[:, pos * D:(pos + 1) * D].T
            BIAS[s * D:(s + 1) * D, 0] = bq[ss]
            BIAS[s * D:(s + 1) * D, 1] = bk[ss]
            BIAS[s * D:(s + 1) * D, 2] = bv[ss]
        cs = slice(c * TCH, (c + 1) * TCH)
        in_maps.append({
            "XT": xT,
            "X1C": np.ascontiguousarray(x1T[:, cs]),
            "X2C": np.ascontiguousarray(x2T[:, cs]),
            "X3C": np.ascontiguousarray(x3T[:, cs]),
            "WQ": WQ2, "WK": WK2, "WV": WV2,
            "WSQ": wsqT, "WSK": wskT, "WSV": wsvT,
            "WP": WP2, "EYE2": eye2, "ONES": ones_row,
            "BIAS": BIAS, "SBIAS": sbias,
        })

    ncA = _get_nc("a")
    resA = run_bass_kernel_spmd(ncA, in_maps, list(range(NC))).results

    outT = np.zeros((C, NT), np.float64)
    for c in range(NC):
        outT += resA[c]["OUTT"]

    if special_pos:
        qsT = np.concatenate([resA[c]["QST"] for c in range(NC)], axis=1)
        ksT = np.concatenate([resA[c]["KST"] for c in range(NC)], axis=1)
        vsT = np.concatenate([resA[c]["VST"] for c in range(NC)], axis=1)
        wps = np.zeros((C, D), f32)
        for i in special_pos:
            wps += W_proj[:, i * D:(i + 1) * D]
        wpsT = np.ascontiguousarray(wps.T)
        in_maps_b = []
        for c in range(NC):
            b = c // (NC // B)
            qc = c % (NC // B)
            vaug = np.concatenate(
                [vsT[:, b * T:(b + 1) * T].T, np.ones((T, 1), f32)], axis=1
            )
            in_maps_b.append({
                "QSC": np.ascontiguousarray(
                    qsT[:, b * T + qc * TCH: b * T + (qc + 1) * TCH]),
                "KS": np.ascontiguousarray(ksT[:, b * T:(b + 1) * T]),
                "VAUG": np.ascontiguousarray(vaug),
                "WPS": wpsT, "ONES": ones_row,
            })
        ncB = _get_nc("b")
        resB = run_bass_kernel_spmd(ncB, in_maps_b, list(range(NC))).results
        for c in range(NC):
            b = c // (NC // B)
            qc = c % (NC // B)
            outT[:, b * T + qc * TCH: b * T + (qc + 1) * TCH] += resB[c]["OUTTS"]

    out = outT.T.astype(f32) + b_proj[None, :]
    return out.reshape(B, T, C).astype(f32)
